# revision 71
# baseline (speedup 1.0000x reference)
"""Trainium2 Bass kernel for nn_AttnFPN (conv pyramid + 4-layer transformer
decoder with banded self-attention + dense cross-attention over a conv memory).

Sharding: 8 cores = 2 batches x 4 window-quarters of the concatenated pyramid
row space (1920 rows). Each core computes the full conv pyramid for its batch,
selects a 512-row window (480 owned rows + 16-row halo each side) via a
one-hot selection matmul, runs all 4 decoder layers on the window (halo
shrink absorbs the banded self-attention's +-4 reach per layer), and emits its
480 owned rows. The host assembles the [B, 256, 1920] output.

On-chip layout is feature-major throughout: activations live as X^T
[d on partitions (2x128 chunks), rows on free dim]. All matmul operands are
bf16 (PE: 1 cycle/row vs 4 for fp32); PSUM accumulation stays fp32. Softmax
runs without max subtraction (scores empirically bounded; exp(-1e9)
underflows to 0 for the band mask). Softmax denominators are produced as
32-row broadcasts by interleaved all-ones columns in the V stationary tiles
(ones written by memset; cross-attn V bias is folded into the output-proj
bias on the host). LayerNorm stats use all-(1/256) stationary matmuls over
feature partitions, with full-width (128-broadcast) var/sqrt/reciprocal; LN
betas are folded into downstream projection biases host-side. The K/V
projections for layer l+1 are software-pipelined into layer l's Act-bound
cross-attention window (and its FFN stall); pyramid-level selection overlaps
the next level's conv via disjoint PSUM banks. Per layer, everything after
self-attention (cross-attn queries, R2/LN2/FFN/LN3, LN1/QTc) is trimmed to
the live column range [4(l+1), 512-4(l+1)] — the halo only exists to feed
the banded self-attention's +-4/layer reach, so those columns are dead for
all later consumers (stale columns stay finite and are never read).
"""
import os
import sys

for _p in ('/opt/trn_rl_repo', os.path.expanduser('~/.axon_site/_ro/trn_rl_repo')):
    if os.path.isdir(_p) and _p not in sys.path:
        sys.path.insert(0, _p)

import ml_dtypes
import numpy as np

import concourse.bass as bass
import concourse.mybir as mybir
import concourse.tile as tile
from concourse import bacc
from concourse.bass_utils import run_bass_kernel_spmd
from concourse.masks import make_identity

F32 = mybir.dt.float32
BF16 = mybir.dt.bfloat16
AF = mybir.ActivationFunctionType
OP = mybir.AluOpType

# problem constants
B, CIN, COUT, T, NLV, NLY, H, DFF, KBAND = 2, 512, 256, 2048, 4, 4, 8, 1024, 9
HD = COUT // H           # 32
RW = 512                 # per-core window rows
OWN = 480
HALO = 16
LVL_SIZES = [1024, 512, 256, 128]
LVL_STARTS = [0, 1024, 1536, 1792]
TOT = 1920
NBLK = TOT // 128        # 15 row-blocks of the concat pyramid
XP = 2056                # padded x length (col j holds x[:, j-1], col 0 = zero)
# self-attn subtiles: (q_start, q_len, k_start) window-local
SUBTILES = [(0, 120, 0), (120, 120, 116), (240, 120, 236), (360, 120, 356), (480, 32, 384)]

# packed per-layer bias/param columns (t_lb)
LB_SA, LB_SAO, LB_CAQ, LB_CAK, LB_CAO = 0, 6, 8, 10, 12
LB_FF1, LB_FF2, LB_G, LB_B3 = 14, 22, 24, 30
LB_W = 32

# ---------------------------------------------------------------------------
# device program
# ---------------------------------------------------------------------------


def _build_nc():
    nc = bacc.Bacc("TRN2", target_bir_lowering=False, debug=False, num_devices=8)

    def din(name, shape, dt=BF16):
        return nc.dram_tensor(name, list(shape), dt, kind="ExternalInput")

    t_x = din("xp", [4, 128, XP])                 # x padded, feature chunks
    t_ssel = din("ssel", [NBLK, 128, RW])         # one-hot selection
    t_pe = din("pe", [128, 2, RW])                # sqrt(C)*0 + PE slice, chunked
    t_smask = din("smask", [5, 128, 256])         # additive self masks per subtile
    # conv weights (lhsT layouts [i-chunk 128, o])
    t_w1x1 = din("w1x1", [4, 128, 256])
    t_wn0 = din("wn0", [3, 4, 128, 256])          # tap, cc, 128, 256
    t_wnk = din("wnk", [3, 3, 2, 128, 256])       # lvl-1, tap, cc, 128, 256
    t_cnb = din("cnb", [128, 10], F32)            # conv_b (2) + neck biases (4x2)
    # per-layer transformer weights
    t_sa_w = din("sa_w", [NLY, 2, 128, 768])      # qkv (q pre-scaled)
    t_sa_ow = din("sa_ow", [NLY, 2, 128, 256])
    t_ca_qw = din("ca_qw", [NLY, 2, 128, 256])    # pre-scaled
    t_ca_kw = din("ca_kw", [NLY, 2, 128, 256])
    t_ca_vw = din("ca_vw", [NLY, 2, 128, 512])    # head-interleaved, ones cols zero
    t_ca_ow = din("ca_ow", [NLY, 2, 128, 256])
    t_ff1w = din("ff1w", [NLY, 2, 128, 1024])
    t_ff2w = din("ff2w", [NLY, 8, 128, 256])
    t_lb = din("lb", [NLY, 128, LB_W], F32)       # packed biases/gammas (betas folded)
    t_out = nc.dram_tensor("out", [128, 2, RW], F32, kind="ExternalOutput")

    with tile.TileContext(nc) as tc:
        _emit(nc, tc, locals())
    nc.compile()
    return nc


def _emit(nc, tc, t):
    from contextlib import ExitStack
    ctx = ExitStack()
    with ctx:
        P = 128
        persist = ctx.enter_context(tc.tile_pool(name="persist", bufs=1))
        state = ctx.enter_context(tc.tile_pool(name="state", bufs=5))
        big = ctx.enter_context(tc.tile_pool(name="big", bufs=2))
        kvp = ctx.enter_context(tc.tile_pool(name="kvp", bufs=2))
        wroll = ctx.enter_context(tc.tile_pool(name="wroll", bufs=2))
        wb = ctx.enter_context(tc.tile_pool(name="wb", bufs=2))
        work = ctx.enter_context(tc.tile_pool(name="work", bufs=2))
        stats = ctx.enter_context(tc.tile_pool(name="stats", bufs=1))
        act = ctx.enter_context(tc.tile_pool(name="act", bufs=1))
        epool = ctx.enter_context(tc.tile_pool(name="epool", bufs=4))
        psc = ctx.enter_context(tc.tile_pool(name="psc", bufs=2, space="PSUM"))
        pav = ctx.enter_context(tc.tile_pool(name="pav", bufs=2, space="PSUM"))
        pms = ctx.enter_context(tc.tile_pool(name="pms", bufs=2, space="PSUM"))

        def stride2(ap3, cc, s, w):
            return ap3[:, cc, s:s + 2 * w].rearrange("p (n two) -> p two n", two=2)[:, 0, :]

        # ---- constants ----
        ident = persist.tile([P, P], F32)
        make_identity(nc, ident[:])
        ident_b = persist.tile([P, P], BF16)
        nc.vector.tensor_copy(ident_b[:], ident[:])
        invn = persist.tile([P, P], BF16)
        nc.gpsimd.memset(invn[:], 1.0 / COUT)
        eps_t = persist.tile([P, 1], F32)
        nc.gpsimd.memset(eps_t[:], 1e-5)

        pe_sb = persist.tile([P, 2, RW], BF16)
        smask_sb = persist.tile([P, 5, 256], BF16)

        memT = persist.tile([P, 2, T], BF16)

        import os as _os
        _nlayers = int(_os.environ.get('KERN_NLAYERS', str(NLY)))
        _stage = _os.environ.get('KERN_STAGE', 'all')

        def kv_steps(l, lb_l):
            """Yield KV-projection work for layer l in small chunks so it can
            be interleaved under Act-bound phases. First yields (KT, Vp)."""
            KT = kvp.tile([P, 2, T], BF16, tag="KT", name=f"KT{l}")
            Vp = kvp.tile([P, 16, 512], BF16, tag="Vp", name=f"Vp{l}")
            wk = wroll.tile([P, 2, 256], BF16, tag="wcak")
            nc.sync.dma_start(out=wk[:], in_=t['t_ca_kw'][l].rearrange("c p f -> p c f"))
            wv = wroll.tile([P, 2, 512], BF16, tag="wv")
            nc.sync.dma_start(out=wv[:], in_=t['t_ca_vw'][l].rearrange("c p f -> p c f"))
            Vp_v = Vp[:].rearrange("p k (h two j) -> p k h two j", h=8, two=2)
            nc.gpsimd.memset(Vp_v[:, :, :, 1, :], 1.0)
            yield (KT, Vp)
            for oc in range(2):
                for q in range(4):
                    ps = pms.tile([P, 512], F32, tag="m")
                    for ic in range(2):
                        nc.tensor.matmul(ps[:], wk[:, ic, 128 * oc:128 * (oc + 1)],
                                         memT[:, ic, 512 * q:512 * (q + 1)],
                                         start=(ic == 0), stop=(ic == 1))
                    nc.vector.tensor_scalar_add(KT[:, oc, 512 * q:512 * (q + 1)],
                                                ps[:], lb_l[:, LB_CAK + oc:LB_CAK + oc + 1])
                    yield None
            for kc in range(16):
                ps = pms.tile([P, 512], F32, tag="m")
                for ic in range(2):
                    nc.tensor.matmul(ps[:], memT[:, ic, 128 * kc:128 * (kc + 1)],
                                     wv[:, ic, :], start=(ic == 0), stop=(ic == 1))
                ps_v = ps[:].rearrange("p (h two j) -> p h two j", h=8, two=2)
                nc.vector.tensor_copy(Vp_v[:, kc, :, 0, :], ps_v[:, :, 0, :])
                if kc % 2 == 1:
                    yield None

        def drain(gen):
            if gen is not None:
                for _ in gen:
                    pass


        # ================= pyramid =================
        with tc.tile_pool(name="pyr", bufs=1) as pyr:
            xT = pyr.tile([P, 4, XP], BF16)
            w1x1 = pyr.tile([P, 4, 256], BF16)
            nc.sync.dma_start(out=xT[:, 0, :], in_=t['t_x'][0])
            nc.sync.dma_start(out=w1x1[:], in_=t['t_w1x1'].ap().rearrange("c p f -> p c f"))
            for cc in range(1, 4):
                nc.sync.dma_start(out=xT[:, cc, :], in_=t['t_x'][cc])
            cnb = pyr.tile([P, 10], F32)
            nc.sync.dma_start(out=cnb[:], in_=t['t_cnb'].ap())
            wn0 = pyr.tile([P, 12, 256], BF16)
            nc.sync.dma_start(out=wn0[:], in_=t['t_wn0'].ap().rearrange("t c p f -> p (t c) f"))
            wnk = pyr.tile([P, 18, 256], BF16)
            nc.sync.dma_start(out=wnk[:], in_=t['t_wnk'].ap().rearrange("l t c p f -> p (l t c) f"))
            ssel = pyr.tile([P, NBLK, RW], BF16)
            nc.sync.dma_start(out=ssel[:], in_=t['t_ssel'].ap().rearrange("b p f -> p b f"))
            lbs_t = []
            for l in range(_nlayers):
                lb_l = wb.tile([P, LB_W], F32, tag="lb", name=f"lb{l}")
                nc.sync.dma_start(out=lb_l[:], in_=t['t_lb'][l])
                lbs_t.append(lb_l)
            kv_gen = kv_steps(0, lbs_t[0]) if _nlayers else None
            kv_tiles = next(kv_gen) if kv_gen else None
            nc.sync.dma_start(out=pe_sb[:], in_=t['t_pe'].ap())
            nc.sync.dma_start(out=smask_sb[:], in_=t['t_smask'].ap().rearrange("s p f -> p s f"))

            # mem = relu(1x1 conv), fc processed in pairs (2 psum slots)
            for oc in range(2):
                for fp in range(2):
                    pss = [pms.tile([P, 512], F32, tag="m", name=f"mempp{oc}_{fp}_{i2}") for i2 in range(2)]
                    for cc in range(4):
                        wsl = w1x1[:, cc, 128 * oc:128 * (oc + 1)]
                        for i, fc in enumerate((2 * fp, 2 * fp + 1)):
                            nc.tensor.matmul(pss[i][:], wsl,
                                             xT[:, cc, 1 + 512 * fc:1 + 512 * (fc + 1)],
                                             start=(cc == 0), stop=(cc == 3))
                    for i, fc in enumerate((2 * fp, 2 * fp + 1)):
                        nc.vector.tensor_scalar(out=memT[:, oc, 512 * fc:512 * (fc + 1)],
                                                in0=pss[i][:], scalar1=cnb[:, oc:oc + 1],
                                                scalar2=0.0, op0=OP.add, op1=OP.max)

            # neck pyramid (feature-major, 1-col zero pad left); each level's
            # transpose+selection blocks are emitted right after the level's
            # conv so they overlap the next level's conv (disjoint PSUM tags:
            # conv='m', transposes='sc', selection accumulators='av')
            lvl_len = [1024, 512, 256, 128]
            sel_ps = [pav.tile([P, 512], F32, tag="av", name=f"selps{dc}") for dc in range(2)]
            f0 = state.tile([P, 2, RW], BF16, tag="fT")
            blk0 = [0, 8, 12, 14]
            src_buf = None
            for lv in range(4):
                L = lvl_len[lv]
                lb_t = pyr.tile([P, 2, L + 8], BF16, tag=f"lb{lv}", name=f"lb{lv}")
                nc.gpsimd.memset(lb_t[:], 0.0)
                n_cc = 4 if lv == 0 else 2
                nfc = (L + 511) // 512
                for oc in range(2):
                    pss = [pms.tile([P, 512], F32, tag="m", name=f"cvp{lv}_{oc}_{i2}") for i2 in range(nfc)]
                    k = 0
                    for cc in range(n_cc):
                        for tap in range(3):
                            if lv == 0:
                                wsl = wn0[:, 4 * tap + cc, 128 * oc:128 * (oc + 1)]
                            else:
                                wsl = wnk[:, 6 * (lv - 1) + 2 * tap + cc, 128 * oc:128 * (oc + 1)]
                            for fc in range(nfc):
                                w = min(512, L - 512 * fc)
                                rhs = (stride2(xT, cc, 1024 * fc + tap, w) if lv == 0
                                       else stride2(src_buf, cc, 1024 * fc + tap, w))
                                nc.tensor.matmul(pss[fc][:, :w], wsl, rhs,
                                                 start=(k == 0), stop=(k == 3 * n_cc - 1))
                            k += 1
                    for fc in range(nfc):
                        w = min(512, L - 512 * fc)
                        nc.vector.tensor_scalar(out=lb_t[:, oc, 1 + 512 * fc:1 + 512 * fc + w],
                                                in0=pss[fc][:, :w], scalar1=cnb[:, 2 + 2 * lv + oc:3 + 2 * lv + oc],
                                                scalar2=0.0, op0=OP.add, op1=OP.max)
                src_buf = lb_t
                for j in range(L // 128):
                    b = blk0[lv] + j
                    for dc in range(2):
                        tr_ps = psc.tile([P, P], BF16, tag="sc")
                        nc.tensor.transpose(tr_ps[:],
                                            lb_t[:, dc, 1 + 128 * j:1 + 128 * (j + 1)],
                                            ident_b[:])
                        fr = work.tile([P, P], BF16, tag="frow")
                        nc.vector.tensor_copy(fr[:], tr_ps[:])
                        nc.tensor.matmul(sel_ps[dc][:], fr[:], ssel[:, b, :],
                                         start=(b == 0), stop=(b == NBLK - 1))
                    if kv_gen is not None:
                        next(kv_gen, None)
            drain(kv_gen)
            kv_gen = None
            for dc in range(2):
                nc.vector.scalar_tensor_tensor(out=f0[:, dc, :], in0=sel_ps[dc][:],
                                               scalar=float(np.sqrt(COUT)),
                                               in1=pe_sb[:, dc, :],
                                               op0=OP.mult, op1=OP.add)


        # ================= decoder layers =================
        fT = f0

        def _unused_kv_steps(l, lb_l):
            """Yield KV-projection work for layer l in small chunks so it can
            be interleaved under the Act-bound cross-attn of layer l-1.
            Yields after every few PE ops; first yields (KT, Vp) tiles."""
            KT = kvp.tile([P, 2, T], BF16, tag="KT", name=f"KT{l}")
            Vp = kvp.tile([P, 16, 512], BF16, tag="Vp", name=f"Vp{l}")
            wk = wroll.tile([P, 2, 256], BF16, tag="wcak")
            nc.sync.dma_start(out=wk[:], in_=t['t_ca_kw'][l].rearrange("c p f -> p c f"))
            wv = wroll.tile([P, 2, 512], BF16, tag="wv")
            nc.sync.dma_start(out=wv[:], in_=t['t_ca_vw'][l].rearrange("c p f -> p c f"))
            Vp_v = Vp[:].rearrange("p k (h two j) -> p k h two j", h=8, two=2)
            nc.gpsimd.memset(Vp_v[:, :, :, 1, :], 1.0)
            yield (KT, Vp)
            for oc in range(2):
                for q in range(4):
                    ps = pms.tile([P, 512], F32, tag="m")
                    for ic in range(2):
                        nc.tensor.matmul(ps[:], wk[:, ic, 128 * oc:128 * (oc + 1)],
                                         memT[:, ic, 512 * q:512 * (q + 1)],
                                         start=(ic == 0), stop=(ic == 1))
                    nc.vector.tensor_scalar_add(KT[:, oc, 512 * q:512 * (q + 1)],
                                                ps[:], lb_l[:, LB_CAK + oc:LB_CAK + oc + 1])
                    yield None
            for kc in range(16):
                ps = pms.tile([P, 512], F32, tag="m")
                for ic in range(2):
                    nc.tensor.matmul(ps[:], memT[:, ic, 128 * kc:128 * (kc + 1)],
                                     wv[:, ic, :], start=(ic == 0), stop=(ic == 1))
                ps_v = ps[:].rearrange("p (h two j) -> p h two j", h=8, two=2)
                nc.vector.tensor_copy(Vp_v[:, kc, :, 0, :], ps_v[:, :, 0, :])
                if kc % 2 == 1:
                    yield None

        def drain(gen):
            if gen is not None:
                for _ in gen:
                    pass

        for l in range(_nlayers):
            lb = lbs_t[l]
            KT, Vp = kv_tiles
            if l + 1 < _nlayers:
                kv_gen = kv_steps(l + 1, lbs_t[l + 1])
                kv_tiles = next(kv_gen)
            else:
                kv_gen = None

            if _stage == 'kv':
                drain(kv_gen)
                continue
            # ---- self attention ----
            wsa = wroll.tile([P, 2, 768], BF16, tag="wsa")
            nc.sync.dma_start(out=wsa[:], in_=t['t_sa_w'][l].rearrange("c p f -> p c f"))
            QTs = act.tile([P, 2, RW], BF16, tag="QTs")
            KTs = act.tile([P, 2, RW], BF16, tag="KTs")
            VTs = act.tile([P, 2, RW], BF16, tag="VTs")
            b0 = 4 * l
            b1 = RW - b0
            bw = b1 - b0
            for wi, dst in ((0, QTs), (1, KTs), (2, VTs)):
                for oc in range(2):
                    ps = pms.tile([P, 512], F32, tag="m")
                    for ic in range(2):
                        nc.tensor.matmul(ps[:, 0:bw], wsa[:, ic, 256 * wi + 128 * oc:256 * wi + 128 * (oc + 1)],
                                         fT[:, ic, b0:b1],
                                         start=(ic == 0), stop=(ic == 1))
                    nc.vector.tensor_scalar_add(dst[:, oc, b0:b1], ps[:, 0:bw],
                                                lb[:, LB_SA + 2 * wi + oc:LB_SA + 2 * wi + oc + 1])
            OsT = act.tile([P, 2, RW], BF16, tag="OT")
            wsao = wroll.tile([P, 2, 256], BF16, tag="wsao")
            nc.sync.dma_start(out=wsao[:], in_=t['t_sa_ow'][l].rearrange("c p f -> p c f"))
            R1 = state.tile([P, 2, RW], BF16, tag="fT")
            for sti, (qs, ql, ks) in enumerate(SUBTILES):
                # vst: transposed V block [keys, 8*32 vdims] + 32 ones cols
                vst = work.tile([P, 288], BF16, tag="vst")
                for hc in range(2):
                    vst_ps = pms.tile([P, 128], BF16, tag="m")
                    nc.tensor.transpose(vst_ps[:],
                                        VTs[:, hc, ks:ks + 128], ident_b[:])
                    nc.vector.tensor_copy(vst[:, 128 * hc:128 * (hc + 1)], vst_ps[:])
                nc.gpsimd.memset(vst[:, 256:288], 1.0)
                # all 8 heads' scores (+mask) in one PSUM tile, one exp
                sps = psc.tile([P, 1024], F32, tag="sc")
                for hh in range(8):
                    nc.tensor.matmul(sps[:, 128 * hh:128 * hh + ql],
                                     KTs[32 * (hh % 4):32 * (hh % 4) + 32, hh // 4, ks:ks + 128],
                                     QTs[32 * (hh % 4):32 * (hh % 4) + 32, hh // 4, qs:qs + ql],
                                     start=True, stop=False,
                                     tile_position=(32 * (hh % 4), 0))
                    nc.tensor.matmul(sps[:, 128 * hh:128 * hh + ql], ident_b[:],
                                     smask_sb[:, sti, 0:ql],
                                     start=False, stop=True)
                es = epool.tile([P, 1024], BF16, tag="E")
                nc.scalar.activation(out=es[:].rearrange("p (h q) -> p h q", h=8)[:, :, 0:ql],
                                     in_=sps[:].rearrange("p (h q) -> p h q", h=8)[:, :, 0:ql],
                                     func=AF.Exp)
                # avp: rows 0:64 = AV (2 heads), 64:128 = denominators; cols 128p
                avp = pav.tile([P, 512], F32, tag="av")
                for p in range(4):
                    h0, h1 = 2 * p, 2 * p + 1
                    nc.tensor.matmul(avp[0:32, 128 * p:128 * p + ql], vst[:, 64 * p:64 * p + 32],
                                     es[:, 128 * h0:128 * h0 + ql], start=True, stop=True,
                                     tile_position=(0, 0))
                    nc.tensor.matmul(avp[32:64, 128 * p:128 * p + ql], vst[:, 64 * p + 32:64 * p + 64],
                                     es[:, 128 * h1:128 * h1 + ql], start=True, stop=True,
                                     tile_position=(0, 32))
                    nc.tensor.matmul(avp[64:96, 128 * p:128 * p + ql], vst[:, 256:288],
                                     es[:, 128 * h0:128 * h0 + ql], start=True, stop=True,
                                     tile_position=(0, 64))
                    nc.tensor.matmul(avp[96:128, 128 * p:128 * p + ql], vst[:, 256:288],
                                     es[:, 128 * h1:128 * h1 + ql], start=True, stop=True,
                                     tile_position=(0, 96))
                zr = work.tile([P, 512], F32, tag="zr")
                nc.vector.reciprocal(zr[64:128, :], avp[64:128, :])
                for p in range(4):
                    nc.vector.tensor_mul(OsT[64 * (p % 2):64 * (p % 2) + 64, p // 2, qs:qs + ql],
                                         avp[0:64, 128 * p:128 * p + ql],
                                         zr[64:128, 128 * p:128 * p + ql])
            a0 = 4 * (l + 1)
            a1 = RW - a0
            aw = a1 - a0
            # columns outside [a0, a1) are halo that no later consumer reads
            # (self-attn band reach shrinks the needed range 4/side per layer)
            for oc in range(2):
                ps = pms.tile([P, 512], F32, tag="m")
                for ic in range(2):
                    nc.tensor.matmul(ps[:, 0:aw], wsao[:, ic, 128 * oc:128 * (oc + 1)], OsT[:, ic, a0:a1],
                                     start=(ic == 0), stop=(ic == 1))
                nc.vector.scalar_tensor_tensor(out=R1[:, oc, a0:a1], in0=ps[:, 0:aw],
                                               scalar=lb[:, LB_SAO + oc:LB_SAO + oc + 1], in1=fT[:, oc, a0:a1],
                                               op0=OP.add, op1=OP.add)
            if kv_gen is not None:
                for _ in range(3):
                    next(kv_gen, None)
            f1 = state.tile([P, 2, RW], BF16, tag="fT")
            _layernorm(nc, pms, work, stats, act, R1, f1, lb, 0, invn, eps_t, a0, a1)

            if _stage == 'self':
                drain(kv_gen)
                fT = f1
                continue
            # ---- cross attention: two q-halves, 4-head quads per exp
            # slice; the A-half post-chain (R2/LN2/FFN/LN3 cols 0:256) is
            # pumped under the B-half's Act-bound window ----
            wcaq = wroll.tile([P, 2, 256], BF16, tag="wcaq")
            nc.sync.dma_start(out=wcaq[:], in_=t['t_ca_qw'][l].rearrange("c p f -> p c f"))
            wcao = wroll.tile([P, 2, 256], BF16, tag="wcao")
            nc.sync.dma_start(out=wcao[:], in_=t['t_ca_ow'][l].rearrange("c p f -> p c f"))
            wf1 = wroll.tile([P, 2, 1024], BF16, tag="wf1")
            nc.sync.dma_start(out=wf1[:], in_=t['t_ff1w'][l].rearrange("c p f -> p c f"))
            wf2 = wroll.tile([P, 8, 256], BF16, tag="wf2")
            nc.sync.dma_start(out=wf2[:], in_=t['t_ff2w'][l].rearrange("c p f -> p c f"))
            QTc = act.tile([P, 2, RW], BF16, tag="QTc")
            for oc in range(2):
                ps = pms.tile([P, 512], F32, tag="m")
                for ic in range(2):
                    nc.tensor.matmul(ps[:, 0:aw], wcaq[:, ic, 128 * oc:128 * (oc + 1)], f1[:, ic, a0:a1],
                                     start=(ic == 0), stop=(ic == 1))
                nc.vector.tensor_scalar_add(QTc[:, oc, a0:a1], ps[:, 0:aw], lb[:, LB_CAQ + oc:LB_CAQ + oc + 1])
            OcT = act.tile([P, 2, RW], BF16, tag="OT")
            R2 = state.tile([P, 2, RW], BF16, tag="fT")
            f2 = state.tile([P, 2, RW], BF16, tag="fT")
            R3 = state.tile([P, 2, RW], BF16, tag="fT")
            f3 = state.tile([P, 2, RW], BF16, tag="fT")

            def emit_r2(c0, c1):
                w = c1 - c0
                for oc in range(2):
                    ps = pms.tile([P, 512], F32, tag="m")
                    for ic in range(2):
                        nc.tensor.matmul(ps[:, 0:w], wcao[:, ic, 128 * oc:128 * (oc + 1)],
                                         OcT[:, ic, c0:c1], start=(ic == 0), stop=(ic == 1))
                    nc.vector.scalar_tensor_tensor(out=R2[:, oc, c0:c1], in0=ps[:, 0:w],
                                                   scalar=lb[:, LB_CAO + oc:LB_CAO + oc + 1],
                                                   in1=f1[:, oc, c0:c1], op0=OP.add, op1=OP.add)

            def ffn_steps(c0, c1):
                """FFN1 -> FFN2 -> R3 for columns [c0, c1). No Act-engine ops,
                so it can pump under cross-attn without table switches."""
                w = c1 - c0
                ps_oc = [pms.tile([P, 512], F32, tag="m", name=f"ffp{l}_{i2}") for i2 in range(2)]
                for hf in range(2):
                    Hb = big.tile([P, 4, 512], BF16, tag="Hb")
                    for jj in range(4):
                        j = 4 * hf + jj
                        ps = psc.tile([P, 1024], F32, tag="sc")
                        for ic in range(2):
                            nc.tensor.matmul(ps[:, 0:w], wf1[:, ic, 128 * j:128 * (j + 1)],
                                             f2[:, ic, c0:c1], start=(ic == 0), stop=(ic == 1))
                        nc.vector.tensor_scalar(out=Hb[:, jj, 0:w], in0=ps[:, 0:w],
                                                scalar1=lb[:, LB_FF1 + j:LB_FF1 + j + 1], scalar2=0.0,
                                                op0=OP.add, op1=OP.max)
                        yield None
                    for jj in range(4):
                        j = 4 * hf + jj
                        for oc in range(2):
                            nc.tensor.matmul(ps_oc[oc][:, 0:w], wf2[:, j, 128 * oc:128 * (oc + 1)],
                                             Hb[:, jj, 0:w], start=(j == 0), stop=(j == 7))
                        yield None
                for oc in range(2):
                    nc.vector.scalar_tensor_tensor(out=R3[:, oc, c0:c1], in0=ps_oc[oc][:, 0:w],
                                                   scalar=lb[:, LB_FF2 + oc:LB_FF2 + oc + 1],
                                                   in1=f2[:, oc, c0:c1], op0=OP.add, op1=OP.add)
                    yield None

            for p in range(4):
                h0, h1 = 2 * p, 2 * p + 1
                avp = pav.tile([P, 512], F32, tag="av")

                def emit_scores(kc):
                    scp = psc.tile([P, 1024], F32, tag="sc")
                    for hi, hh in enumerate((h0, h1)):
                        nc.tensor.matmul(scp[:, 512 * hi:512 * hi + aw],
                                         KT[32 * (hh % 4):32 * (hh % 4) + 32, hh // 4, 128 * kc:128 * (kc + 1)],
                                         QTc[32 * (hh % 4):32 * (hh % 4) + 32, hh // 4, a0:a1],
                                         start=True, stop=True, tile_position=(32 * (hh % 4), 0))
                    return scp

                # software-pipelined: scores(kc+1) is emitted before AV(kc),
                # which waits on exp(kc) — keeps PE's in-order queue from
                # stalling the next exp's input
                scp_cur = emit_scores(0)
                for kc in range(16):
                    ec = epool.tile([P, 1024], BF16, tag="E")
                    nc.scalar.activation(out=ec[:].rearrange("p (b q) -> p b q", b=2)[:, :, 0:aw],
                                         in_=scp_cur[:].rearrange("p (b q) -> p b q", b=2)[:, :, 0:aw],
                                         func=AF.Exp)
                    if kc + 1 < 16:
                        scp_nxt = emit_scores(kc + 1)
                    st, sp = (kc == 0), (kc == 15)
                    nc.tensor.matmul(avp[0:64, 0:aw], Vp[:, kc, 64 * h0:64 * h0 + 64],
                                     ec[:, 0:aw], start=st, stop=sp, tile_position=(0, 0))
                    nc.tensor.matmul(avp[64:128, 0:aw], Vp[:, kc, 64 * h1:64 * h1 + 64],
                                     ec[:, 512:512 + aw], start=st, stop=sp, tile_position=(0, 64))
                    if kv_gen is not None and kc % 8 == 7:
                        next(kv_gen, None)
                    if kc + 1 < 16:
                        scp_cur = scp_nxt
                zr = work.tile([P, 512], F32, tag="zr")
                nc.vector.reciprocal(zr[:, 0:aw], avp[:, 0:aw])
                nc.vector.tensor_mul(OcT[64 * (p % 2):64 * (p % 2) + 32, p // 2, a0:a1],
                                     avp[0:32, 0:aw], zr[32:64, 0:aw])
                nc.vector.tensor_mul(OcT[64 * (p % 2) + 32:64 * (p % 2) + 64, p // 2, a0:a1],
                                     avp[64:96, 0:aw], zr[96:128, 0:aw])
            emit_r2(a0, a1)
            _layernorm(nc, pms, work, stats, act, R2, f2, lb, 1, invn, eps_t, a0, a1)
            if kv_gen is not None:
                for _ in range(5):
                    next(kv_gen, None)
            for _ in ffn_steps(a0, a1):
                pass
            drain(kv_gen)
            _layernorm(nc, pms, work, stats, act, R3, f3, lb, 2, invn, eps_t, a0, a1)
            fT = f3

        # final: add back LN3 beta (unfolded) and emit fp32
        outf = act.tile([P, 2, RW], F32, tag="outf")
        for oc in range(2):
            nc.vector.tensor_scalar_add(outf[:, oc, HALO:RW - HALO], fT[:, oc, HALO:RW - HALO],
                                        lb[:, LB_B3 + oc:LB_B3 + oc + 1])
            nc.sync.dma_start(out=t['t_out'].ap()[:, oc, HALO:RW - HALO],
                              in_=outf[:, oc, HALO:RW - HALO])



def _layernorm(nc, pms, work, stats, act, R, out, lb, which, invn, eps_t, c0=0, c1=RW,
               cout=None):
    """Feature-major LN over d=256 (2 partition chunks), rows c0:c1 on the
    free dim. Stats via all-(1/256) stationary matmuls producing 128-row
    broadcasts; gamma per-partition; beta folded into downstream biases.
    If cout is given, the centered residual (R - mean, bf16) is stored there
    so downstream matmuls can start before the rstd is ready (the per-row
    rstd commutes through a matmul's free dim). Returns the rstd tile."""
    P = 128
    w = c1 - c0
    sq = act.tile([P, 2, RW], BF16, tag="sq")
    for oc in range(2):
        nc.vector.tensor_mul(sq[:, oc, c0:c1], R[:, oc, c0:c1], R[:, oc, c0:c1])
    mB = pms.tile([P, 512], F32, tag="m")
    for ic in range(2):
        nc.tensor.matmul(mB[:, 0:w], invn[:], R[:, ic, c0:c1], start=(ic == 0), stop=(ic == 1))
    msB = pms.tile([P, 512], F32, tag="m")
    for ic in range(2):
        nc.tensor.matmul(msB[:, 0:w], invn[:], sq[:, ic, c0:c1], start=(ic == 0), stop=(ic == 1))
    sqm = stats.tile([P, RW], F32, tag="s1")
    nc.scalar.activation(out=sqm[:, 0:w], in_=mB[:, 0:w], func=AF.Square)
    varB = stats.tile([P, RW], F32, tag="s2")
    nc.vector.tensor_sub(varB[:, 0:w], msB[:, 0:w], sqm[:, 0:w])
    sdB = stats.tile([P, RW], F32, tag="s1")
    nc.scalar.activation(out=sdB[:, 0:w], in_=varB[:, 0:w], func=AF.Sqrt, bias=eps_t[:])
    zrB = stats.tile([P, RW], F32, tag="s2")
    nc.vector.reciprocal(zrB[:, 0:w], sdB[:, 0:w])
    for oc in range(2):
        # out = ((R - mB) * gamma) * zrB   (gamma per-partition; beta folded)
        c = work.tile([P, RW], F32, tag="tmp")
        nc.vector.tensor_sub(c[:, 0:w], R[:, oc, c0:c1], mB[:, 0:w])
        nc.vector.scalar_tensor_tensor(out=out[:, oc, c0:c1], in0=c[:, 0:w],
                                       scalar=lb[:, LB_G + 2 * which + oc:LB_G + 2 * which + oc + 1],
                                       in1=zrB[:, 0:w], op0=OP.mult, op1=OP.mult)
    return zrB


# ---------------------------------------------------------------------------
# host side
# ---------------------------------------------------------------------------

def _sinusoidal_pe(t, d):
    pos = np.arange(t, dtype=np.float32)[:, None]
    div = np.exp(np.arange(0, d, 2, dtype=np.float32) * (-np.log(10000.0) / d))
    ang = pos * div
    pe = np.zeros((t, d), np.float32)
    pe[:, 0::2] = np.sin(ang)
    pe[:, 1::2] = np.cos(ang)
    return pe


def _concat_row_to_level(r):
    for li in range(NLV):
        if r < LVL_STARTS[li] + LVL_SIZES[li]:
            return li, r - LVL_STARTS[li]
    raise ValueError(r)


def _core_meta(c):
    w0 = OWN * c - HALO
    S = np.zeros((TOT, RW), np.float32)
    valid = np.zeros(RW, bool)
    lvl_of = np.full(RW, -1)
    pos_of = np.full(RW, -1)
    for j in range(RW):
        r = w0 + j
        if 0 <= r < TOT:
            S[r, j] = 1.0
            valid[j] = True
            lvl_of[j], pos_of[j] = _concat_row_to_level(r)
    pes = [_sinusoidal_pe(sz, COUT) for sz in LVL_SIZES]
    pe_plus = np.zeros((COUT, RW), np.float32)
    for j in range(RW):
        if valid[j]:
            pe_plus[:, j] = pes[lvl_of[j]][pos_of[j]]
    smask = np.full((5, 128, 256), -1e9, np.float32)
    for sti, (qs, ql, ks) in enumerate(SUBTILES):
        m = np.full((128, ql), -1e9, np.float32)
        for jq in range(ql):
            q = qs + jq
            for jk in range(128):
                k = ks + jk
                if k >= RW:
                    continue
                if valid[q] and valid[k]:
                    if lvl_of[q] == lvl_of[k] and abs(pos_of[q] - pos_of[k]) <= KBAND // 2:
                        m[jk, jq] = 0.0
                elif (not valid[q]) and k == q:
                    m[jk, jq] = 0.0
        smask[sti, :, 0:ql] = m
        smask[sti, :, 128:128 + ql] = m
    return S, pe_plus, smask


def _chunk_p(v):
    """[n*128] -> [128, n] partition-major."""
    v = np.asarray(v, np.float32)
    n = v.shape[0] // 128
    return v.reshape(n, 128).T.copy()


def _lhsT(w):
    """[O, I] weight -> [n_ic, 128, O] lhsT chunks (W^T chunked over I)."""
    wT = np.ascontiguousarray(np.asarray(w, np.float32).T)  # [I, O]
    I = wT.shape[0]
    return wT.reshape(I // 128, 128, wT.shape[1])


_NC_CACHE = None
LAST_EXEC_NS = None


def _get_nc():
    global _NC_CACHE
    if _NC_CACHE is None:
        _NC_CACHE = _build_nc()
    return _NC_CACHE


def _bf(a):
    return np.asarray(a, np.float32).astype(ml_dtypes.bfloat16)


def _prepare_in_maps(inputs):
    inp = {k: np.asarray(v, np.float32) for k, v in inputs.items()}

    scale = 1.0 / np.sqrt(HD)
    common = {}
    common['w1x1'] = _bf(_lhsT(inp['conv_w'][:, :, 0]))
    common['wn0'] = _bf(np.stack([_lhsT(inp['neck_w0'][:, :, tp]) for tp in range(3)]))
    common['wnk'] = _bf(np.stack([np.stack([_lhsT(inp['neck_w'][lv][:, :, tp]) for tp in range(3)])
                                  for lv in range(3)]))
    cnb = np.zeros((128, 10), np.float32)
    cnb[:, 0:2] = _chunk_p(inp['conv_b'])
    cnb[:, 2:4] = _chunk_p(inp['neck_b0'])
    for i in range(3):
        cnb[:, 4 + 2 * i:6 + 2 * i] = _chunk_p(inp['neck_b'][i])
    common['cnb'] = cnb

    sa_w = []
    lbs = np.zeros((NLY, 128, LB_W), np.float32)
    for l in range(NLY):
        w = inp['sa_in_w'][l].copy()    # [768, 256]
        b = inp['sa_in_b'][l].copy()
        w[:COUT] *= scale
        b[:COUT] *= scale
        beta3_prev = inp['ln3_b'][l - 1] if l > 0 else np.zeros(COUT, np.float32)
        beta1 = inp['ln1_b'][l]
        beta2 = inp['ln2_b'][l]
        # qkv bias + W_qkv @ beta3(prev layer)   (fT input lacks beta3)
        b_eff = b + w @ beta3_prev
        sa_w.append(_lhsT(w))           # [2, 128, 768]
        for wi in range(3):
            lbs[l, :, LB_SA + 2 * wi:LB_SA + 2 * wi + 2] = _chunk_p(b_eff[wi * COUT:(wi + 1) * COUT])
        lbs[l, :, LB_SAO:LB_SAO + 2] = _chunk_p(inp['sa_out_b'][l] + beta3_prev)
        wq_ca = inp['ca_in_w'][l][:COUT] * scale
        lbs[l, :, LB_CAQ:LB_CAQ + 2] = _chunk_p(inp['ca_in_b'][l][:COUT] * scale + wq_ca @ beta1)
        lbs[l, :, LB_CAK:LB_CAK + 2] = _chunk_p(inp['ca_in_b'][l][COUT:2 * COUT])
        # out bias + folded V bias + beta1 (residual f1 lacks beta1)
        lbs[l, :, LB_CAO:LB_CAO + 2] = _chunk_p(
            inp['ca_out_b'][l] + inp['ca_out_w'][l] @ inp['ca_in_b'][l][2 * COUT:] + beta1)
        lbs[l, :, LB_FF1:LB_FF1 + 8] = _chunk_p(inp['ff1_b'][l] + inp['ff1_w'][l] @ beta2)
        lbs[l, :, LB_FF2:LB_FF2 + 2] = _chunk_p(inp['ff2_b'][l] + beta2)
        for wi, g in enumerate((inp['ln1_g'][l], inp['ln2_g'][l], inp['ln3_g'][l])):
            lbs[l, :, LB_G + 2 * wi:LB_G + 2 * wi + 2] = _chunk_p(g)
        lbs[l, :, LB_B3:LB_B3 + 2] = _chunk_p(inp['ln3_b'][l])
    common['lb'] = lbs
    common['sa_w'] = _bf(np.stack(sa_w))
    common['sa_ow'] = _bf(np.stack([_lhsT(inp['sa_out_w'][l]) for l in range(NLY)]))
    common['ca_qw'] = _bf(np.stack([_lhsT(inp['ca_in_w'][l][:COUT] * scale) for l in range(NLY)]))
    common['ca_kw'] = _bf(np.stack([_lhsT(inp['ca_in_w'][l][COUT:2 * COUT]) for l in range(NLY)]))
    ca_vw = []
    for l in range(NLY):
        wT = _lhsT(inp['ca_in_w'][l][2 * COUT:])          # [2, 128, 256]
        waug = np.zeros((2, 128, 512), np.float32)
        for hh2 in range(H):
            waug[:, :, 64 * hh2:64 * hh2 + 32] = wT[:, :, 32 * hh2:32 * hh2 + 32]
        ca_vw.append(waug)
    common['ca_vw'] = _bf(np.stack(ca_vw))
    common['ca_ow'] = _bf(np.stack([_lhsT(inp['ca_out_w'][l]) for l in range(NLY)]))
    common['ff1w'] = _bf(np.stack([_lhsT(inp['ff1_w'][l]) for l in range(NLY)]))
    common['ff2w'] = _bf(np.stack([_lhsT(inp['ff2_w'][l]) for l in range(NLY)]))

    metas = [_core_meta(c) for c in range(4)]
    in_maps = []
    for core in range(8):
        b, c = core // 4, core % 4
        S, pe_plus, smask = metas[c]
        xp = np.zeros((CIN, XP), np.float32)
        xp[:, 1:1 + T] = inp['x'][b]
        m = dict(common)
        m['xp'] = _bf(xp.reshape(4, 128, XP))
        m['ssel'] = _bf(S.reshape(NBLK, 128, RW))
        m['pe'] = _bf(pe_plus.reshape(2, 128, RW).transpose(1, 0, 2))
        m['smask'] = _bf(smask)
        in_maps.append(m)
    return in_maps


def kernel(**inputs):
    nc = _get_nc()
    in_maps = _prepare_in_maps(inputs)

    global LAST_EXEC_NS
    trace = bool(int(os.environ.get('KERN_TRACE', '0')))
    res = run_bass_kernel_spmd(nc, in_maps, list(range(8)), trace=trace)
    if res.exec_time_ns is not None:
        LAST_EXEC_NS = res.exec_time_ns

    out = np.zeros((B, COUT, TOT), np.float32)
    for core in range(8):
        b, c = core // 4, core % 4
        o = res.results[core]['out']          # [128, 2, RW]
        fT = o.transpose(1, 0, 2).reshape(COUT, RW)
        out[b, :, OWN * c:OWN * (c + 1)] = fT[:, HALO:HALO + OWN]
    return out


def timeline_estimate():
    """Cost-model single-core timeline estimate (ns)."""
    from concourse.timeline_sim import TimelineSim
    nc = _get_nc()
    ts = TimelineSim(nc, trace=False)
    ts.simulate()
    return ts
